# revision 5
# baseline (speedup 1.0000x reference)
"""Trainium2 Bass kernel for 4-D valid convolution (Winograd F(2,3) in z).

Problem: inputs [2, 64, 18, 18, 18, 18] fp32, kernel [81, 64, 64] fp32
(81 = 3^4 offsets row-major over (dw, dx, dy, dz)), output
[2, 64, 16, 16, 16, 16] fp32.

Sharding (8 cores): batch (2) x output-W chunks (4 chunks of 4).  Each core
gets input slabs x[b, :, w0:w0+6] plus the full kernel, and produces
out[b, :, w0:w0+4] as [64, 4, 16, 16, 16].

The PE moving-operand fetch is byte-bandwidth-limited (~450 B/cycle
aggregate across quadrant streams, measured), so runtime scales with
contraction-bytes x output-positions.  Winograd F(2,3) along z cuts that
by 1/3: the host transforms the input into 4 m-copies per z-tile of 2
(m0 = x0-x2, m1 = x1+x2, m2 = x2-x1, m3 = x1-x3) and the weights into
Gw_k (Gw0 = w[dz=0], Gw1 = (w0+w1+w2)/2, Gw2 = (w0-w1+w2)/2, Gw3 =
w[dz=2]); the PE contracts only over (dw,dx,dy) x cin = 27*64 per
m-point, and the epilogue applies the 2-tap inverse (out_even =
M0+M1+M2, out_odd = M1-M2-M3) with the ACT/DVE bank-combine it already
needed.

Layout per (k, slab): m_k[ci, X(18), Y(18), zt(8)], col = X*144 + Y*16/2
... = X*144 + Y*8 + zt.  SBUF dup-tile D_k: rows 64-127 = m_k, rows
0-63 = m_k shifted +8 cols (one y-row), so a K=128 matmul at column q
covers (dy, dy+1) pairs; dy=2 singles run K=64 4-way packed (tile A on
quadrant (0,0) reading lo rows, tile B on (64,64) reading hi rows,
streaming concurrently).

Tiles: out x-range of 4 planes per tile, two tiles (x0, x0+4) in PE
col-groups 0/64 -> 8 tile-pairs (4 w x 2 x-halves), N=512 streams
(4x*16y*8zt), 4 k-accumulations in two 2-bank PSUM tiles per tile-pair.
Loads are column-pieced (x-planes 0-9 / 10-17) and issued in need order
round-robin over the sync/scalar/gpsimd DMA rings; tile-pair loop is
x-half-major so each half only needs its piece of every slab.
"""

import os
import sys

import numpy as np

if "/opt/trn_rl_repo" not in sys.path:
    sys.path.insert(0, "/opt/trn_rl_repo")
os.environ.setdefault("JAX_PLATFORMS", "axon,cpu")

B, CIN, COUT = 2, 64, 64
S = 18          # input spatial per dim
SO = 16         # output spatial per dim
NW = 4          # output w per core
NSLAB = 6      # input w slabs per core
ZT = 8          # z-tiles (of 2) per output
XPL = S * ZT              # 144 cols per x-plane
DC = S * XPL              # 2592 m-cols per slab
DKC = DC + 8              # D_k dram cols incl the +8 (one y-row) shift
DSL = DKC + 16            # D_k sbuf cols incl rearrange-view slack
P01 = 10 * XPL            # piece split: x-planes 0-9 | 10-17

_CACHE = {}


def _build_nc(dt_in):
    import concourse.bass as bass
    import concourse.mybir as mybir

    f32 = mybir.dt.float32

    nc = bass.Bass()
    d_h = [
        nc.dram_tensor(f"d{k}", [128, NSLAB, DKC], dt_in, kind="ExternalInput")
        for k in range(4)
    ]
    # wp{k}: pair weights, lo rows = Gw_k[(dw,dx), dy=0], hi rows = dy=1
    # ws{k}: single weights, both halves = Gw_k[(dw,dx), dy=2]
    wp_h = [
        nc.dram_tensor(f"wp{k}", [128, 9, COUT], dt_in, kind="ExternalInput")
        for k in range(4)
    ]
    ws_h = [
        nc.dram_tensor(f"ws{k}", [128, 9, COUT], dt_in, kind="ExternalInput")
        for k in range(4)
    ]
    out_h = nc.dram_tensor(
        "out", [COUT, NW, SO, SO, SO], f32, kind="ExternalOutput"
    )

    tc = _make_tile_context(nc)
    with tc:
        with (
            tc.tile_pool(name="xp", bufs=1) as xpool,
            tc.tile_pool(name="wpl", bufs=1) as wpool,
            tc.tile_pool(name="ob", bufs=3) as opool,
            tc.tile_pool(name="ps", bufs=2, space="PSUM") as ppool,
        ):
            # scalar (ACT) issues no DMA: trigger instructions stall on
            # DMA-ring backpressure in the engine FIFO, which would delay
            # the epilogue ACTIVATEs behind them (and the PSUM bank frees
            # the next tile-pair waits on).
            dma_engines = [nc.sync, nc.gpsimd]
            dma_rr = [0]

            def dma(dst, src):
                dma_engines[dma_rr[0] % 2].dma_start(dst, src)
                dma_rr[0] += 1

            wps, wss = [], []
            for k in range(4):
                wp_t = wpool.tile([128, 9, COUT], dt_in, tag=f"wp{k}")
                ws_t = wpool.tile([128, 9, COUT], dt_in, tag=f"ws{k}")
                wps.append(wp_t)
                wss.append(ws_t)
            ds = []
            for k in range(4):
                row = []
                for s in range(NSLAB):
                    d_t = xpool.tile([128, DSL], dt_in, tag=f"d{k}s{s}")
                    row.append(d_t)
                ds.append(row)

            # need-ordered loads: the first tile-pair consumes k-phases in
            # order, each reading slabs 0-2 of its k, so interleave
            # [weights_k, D_k slabs 0-2] per k, then the w>0 slabs, then
            # the second x-half pieces.  The k=0 gate data is split in
            # column halves so both rings transfer it in parallel.
            def dma2(dst, src, n):
                h = n // 2
                dma(dst[:, 0:h], src[:, 0:h])
                dma(dst[:, h:n], src[:, h:n])

            for k in range(4):
                if k == 0:
                    dma(wps[0][:], wp_h[0][:])
                    dma(wss[0][:], ws_h[0][:])
                    for s in range(3):
                        dma2(ds[0][s], d_h[0][:, s], P01)
                else:
                    dma(wps[k][:], wp_h[k][:])
                    dma(wss[k][:], ws_h[k][:])
                    for s in range(3):
                        dma(ds[k][s][:, 0:P01], d_h[k][:, s, 0:P01])
            for s in range(3, NSLAB):
                for k in range(4):
                    dma(ds[k][s][:, 0:P01], d_h[k][:, s, 0:P01])
            for s in range(NSLAB):
                for k in range(4):
                    dma(ds[k][s][:, P01:DKC], d_h[k][:, s, P01:DKC])

            # (no HAM warmup: the engine preamble ends ~7.2us and the first
            # loads land by ~5us, so scratch warmup matmuls only delay the
            # real stream - the clock ramps on real work instead.)

            def rhs(t, prange, q0, nx=4):
                v = t[prange, q0 : q0 + nx * XPL]
                v = v.rearrange("p (x y z) -> p x y z", x=nx, y=S, z=ZT)
                return v[:, :, 0:16, :]

            PFULL = slice(0, 128)
            PLO = slice(0, 64)
            PHI = slice(64, 128)

            import concourse.mybir as mybir

            def emit_k(pk, k, w, x0, cs, xo, nx):
                # matmul stream for phase k over psum col-slice cs
                # (moving cols = x-planes x0+xo .. x0+xo+nx-1)
                for j2 in range(9):
                    dw, dx = j2 // 3, j2 % 3
                    dt_ = ds[k][w + dw]
                    q = (x0 + xo + dx) * XPL + 8
                    st = j2 == 0
                    nc.tensor.matmul(
                        pk[k][0:64, cs],
                        wps[k][:, j2, :],
                        rhs(dt_, PFULL, q, nx),
                        start=st, stop=False,
                        tile_position=(0, 0),
                    )
                    nc.tensor.matmul(
                        pk[k][64:128, cs],
                        wps[k][:, j2, :],
                        rhs(dt_, PFULL, q + 4 * XPL, nx),
                        start=st, stop=False,
                        tile_position=(0, 64),
                    )
                # dy=2 singles: K=64, 4-way packed (A lo / B hi)
                for j2 in range(9):
                    dw, dx = j2 // 3, j2 % 3
                    dt_ = ds[k][w + dw]
                    last = j2 == 8
                    nc.tensor.matmul(
                        pk[k][0:64, cs],
                        wss[k][0:64, j2, :],
                        rhs(dt_, PLO, (x0 + xo + dx) * XPL + 24, nx),
                        start=False, stop=last,
                        tile_position=(0, 0),
                    )
                    nc.tensor.matmul(
                        pk[k][64:128, cs],
                        wss[k][64:128, j2, :],
                        rhs(dt_, PHI, (x0 + 4 + xo + dx) * XPL + 16, nx),
                        start=False, stop=last,
                        tile_position=(64, 64),
                    )

            def emit_epilogue_half(osb, p01, p23, h0):
                # Winograd inverse along z (even = M0+M1+M2, odd =
                # M1-M2-M3) via ACT copy (psum->sbuf) + DVE ops (single
                # PSUM operand per op)
                hs = slice(h0, h0 + 256)
                evh = osb[:, hs, 0]
                odh = osb[:, hs, 1]
                nc.scalar.copy(evh, p01[:, 0, hs])
                nc.vector.tensor_add(out=evh, in0=p01[:, 1, hs], in1=evh)
                nc.vector.tensor_add(out=evh, in0=p23[:, 0, hs], in1=evh)
                nc.scalar.copy(odh, p01[:, 1, hs])
                nc.vector.scalar_tensor_tensor(
                    out=odh, in0=p23[:, 0, hs], scalar=-1.0, in1=odh,
                    op0=mybir.AluOpType.mult, op1=mybir.AluOpType.add,
                )
                nc.vector.scalar_tensor_tensor(
                    out=odh, in0=p23[:, 1, hs], scalar=-1.0, in1=odh,
                    op0=mybir.AluOpType.mult, op1=mybir.AluOpType.add,
                )

            def emit_stores(osb, w, x0, xo, nx):
                lo = osb[0:64].rearrange(
                    "p (x y zt) r -> p x y (zt r)", x=4, y=16, zt=ZT
                )
                hi = osb[64:128].rearrange(
                    "p (x y zt) r -> p x y (zt r)", x=4, y=16, zt=ZT
                )
                # stores on the HWDGE (sync) ring only: SWDGE stores
                # would hold up the gpsimd teardown drain ~2us.
                for c0 in range(xo, xo + nx, 2):
                    nc.sync.dma_start(
                        out_h[:, w, x0 + c0 : x0 + c0 + 2, :, :],
                        lo[:, c0 : c0 + 2],
                    )
                    nc.sync.dma_start(
                        out_h[:, w, x0 + 4 + c0 : x0 + 6 + c0, :, :],
                        hi[:, c0 : c0 + 2],
                    )

            # ---- main loop: 8 tile-pairs, x-half-major ----
            for xh in range(2):
                x0 = 8 * xh       # tile A covers x-planes x0..x0+3
                for w in range(NW):
                    p01 = ppool.tile([128, 2, 512], f32, tag="p01")
                    p23 = ppool.tile([128, 2, 512], f32, tag="p23")
                    pk = [p01[:, 0], p01[:, 1], p23[:, 0], p23[:, 1]]
                    osb = opool.tile([128, 512, 2], f32, tag="osb")

                    if xh == 1 and w == NW - 1:
                        # last tile-pair: run per column-half (N=256) so
                        # the half-0 epilogue+stores hide under half-1's
                        # matmul stream, shrinking the exposed tail.
                        for h in range(2):
                            for k in range(4):
                                emit_k(pk, k, w, x0,
                                       slice(256 * h, 256 * h + 256), 2 * h, 2)
                            emit_epilogue_half(osb, p01, p23, 256 * h)
                            emit_stores(osb, w, x0, 2 * h, 2)
                    else:
                        for k in range(4):
                            emit_k(pk, k, w, x0, slice(0, 512), 0, 4)
                        # column-halved so ACT and DVE pipeline (halves
                        # the exposed epilogue latency + PSUM-free delay)
                        for h0 in (0, 256):
                            emit_epilogue_half(osb, p01, p23, h0)
                        emit_stores(osb, w, x0, 0, 4)

    _split_multiwaits(nc)
    return nc


def _make_tile_context(nc):
    from concourse.tile import TileContext

    class TC(TileContext):
        # stock teardown is drain -> barrier -> sem-clear -> barrier; the
        # final barrier only orders engine-stream ends and costs ~2us.
        def _drain_and_barrier(self, tick_clock, wait_clock):
            from concourse.vector_clock import ScopedClock

            nc = self.nc
            drain_inst = nc.sync.drain()
            wait_clock.add_sem_waits(
                drain_inst.ins, ScopedClock({None: tick_clock.global_clock})
            )
            # mark for _split_multiwaits: distribute this drain's waits
            # round-robin across all engines (parallel ~10 NoOps each)
            # instead of ~60 serial NoOps on sync (~1.5us tail).  The
            # barrier right after orders every NoOp before the sem clear.
            nc._final_drain_name = drain_inst.ins.name
            nc.all_engine_barrier()
            assert self.sems is not None
            popped = nc._tile_sem_poison_stack.pop()
            assert popped is self._sem_poison
            nc.clear_and_free_semaphores(list(self.sems.allocated().values()))

    return TC(nc)


def _split_multiwaits(nc, max_waits=1):
    """The walrus build here rejects any instruction carrying more than one
    sync-wait ("Too many sync wait commands").  Tile attaches one wait per
    outstanding producer.  Move excess waits onto same-engine NoOps inserted
    immediately before the instruction - semantically identical."""
    import concourse.mybir as mybir

    final_drain = getattr(nc, "_final_drain_name", None)
    engines = list(nc.engines)
    n_split = 0
    for fn in nc.m.functions:
        for blk in fn.blocks:
            out = []
            for inst in list(blk.instructions):
                si = inst.sync_info
                if si is not None and si.on_wait and len(si.on_wait) > max_waits:
                    waits = list(si.on_wait)
                    extra = waits[:-max_waits]
                    spread = inst.name == final_drain
                    for k in range(0, len(extra), max_waits):
                        nop = mybir.InstNoOp(
                            name=f"{inst.name}.w{k}", ins=[], outs=[]
                        )
                        if spread:
                            nop.engine = engines[(k // max_waits) % len(engines)]
                        else:
                            nop.engine = inst.engine
                        nop.sync_info = mybir.SyncInfo(
                            on_wait=extra[k : k + max_waits], on_update=[]
                        )
                        nc.register_instruction(nop)
                        out.append(nop)
                        n_split += 1
                    si.on_wait = waits[-max_waits:]
                out.append(inst)
            blk.instructions = out
    return n_split


# compute dtype: "float16" (fastest, rel err ~4e-4) or "float32r"
DTYPE = "float16"


def _get_nc():
    if "nc" not in _CACHE:
        import concourse.mybir as mybir

        _CACHE["nc"] = _build_nc(getattr(mybir.dt, DTYPE))
    return _CACHE["nc"]


def _np_dtype():
    if DTYPE == "float16":
        return np.float16
    return np.float32


def _shard_inputs(inputs):
    nd = _np_dtype()
    x = np.asarray(inputs["inputs"], dtype=np.float32)
    wk = np.asarray(inputs["kernel"], dtype=np.float32)
    k5 = wk.reshape(3, 3, 3, 3, CIN, COUT)  # [dw, dx, dy, dz, ci, co]
    # weight transform Gw_k over dz
    w0, w1, w2 = k5[:, :, :, 0], k5[:, :, :, 1], k5[:, :, :, 2]
    gw = [w0, (w0 + w1 + w2) * 0.5, (w0 - w1 + w2) * 0.5, w2]
    wps, wss = [], []
    for k in range(4):
        g = gw[k].reshape(9, 3, CIN, COUT)  # [(dw,dx), dy, ci, co]
        wp = np.concatenate(
            [g[:, 0].transpose(1, 0, 2), g[:, 1].transpose(1, 0, 2)], axis=0
        )
        w2h = g[:, 2].transpose(1, 0, 2)
        ws_ = np.concatenate([w2h, w2h], axis=0)
        wps.append(np.ascontiguousarray(wp.astype(nd)))
        wss.append(np.ascontiguousarray(ws_.astype(nd)))
    in_maps = []
    for c in range(8):
        b, wc = c // 4, c % 4
        w0c = 4 * wc
        sl = x[b, :, w0c : w0c + 6]             # [CIN, 6, 18, 18, 18] fp32
        ze = sl[..., 0::2]                      # z even: 0,2,..,16 (9)
        zo = sl[..., 1::2]                      # z odd: 1,3,..,17 (9)
        # m_k[ci, s, X, Y, zt], zt = 0..7: windows z = 2zt .. 2zt+3
        m = [
            ze[..., 0:8] - ze[..., 1:9],        # x0 - x2
            zo[..., 0:8] + ze[..., 1:9],        # x1 + x2
            ze[..., 1:9] - zo[..., 0:8],        # x2 - x1
            zo[..., 0:8] - zo[..., 1:9],        # x1 - x3
        ]
        feeds = {}
        for k in range(4):
            mk = m[k].reshape(CIN, NSLAB, DC).astype(nd)
            dk = np.zeros((128, NSLAB, DKC), dtype=nd)
            dk[0:CIN, :, 8:DKC] = mk            # lo rows: m_k[c-8]
            dk[CIN:, :, 0:DC] = mk              # hi rows: m_k[c]
            feeds[f"d{k}"] = dk
            feeds[f"wp{k}"] = wps[k]
            feeds[f"ws{k}"] = wss[k]
        in_maps.append(feeds)
    return in_maps


def _gather_outputs(results):
    out = np.empty((B, COUT, NW * 4, SO, SO, SO), dtype=np.float32)
    for c in range(8):
        b, wc = c // 4, c % 4
        w0 = 4 * wc
        out[b, :, w0 : w0 + 4] = results[c]["out"]
    return out


def kernel(**inputs):
    from concourse.bass_utils import run_bass_kernel_spmd

    res = run_bass_kernel_spmd(_get_nc(), _shard_inputs(inputs), list(range(8)))
    return _gather_outputs(res.results)



# revision 7
# speedup vs baseline: 1.1397x; 1.1397x over previous
"""Trainium2 Bass kernel for 4-D valid convolution (Winograd F(4,3) in z).

Problem: inputs [2, 64, 18, 18, 18, 18] fp32, kernel [81, 64, 64] fp32
(81 = 3^4 offsets row-major over (dw, dx, dy, dz)), output
[2, 64, 16, 16, 16, 16] fp32.

Sharding (8 cores): batch (2) x output-W chunks (4 chunks of 4).  Each core
gets input slabs x[b, :, w0:w0+6] plus the full kernel, and produces
out[b, :, w0:w0+4] as [64, 4, 16, 16, 16].

The PE is column-issue limited: every matmul step costs ~N cycles at
2.4 GHz regardless of K, with two quadrant streams running concurrently
(col-groups 0/64).  Runtime therefore scales with (#steps x N) =
m-columns per output.  Winograd F(4,3) along z (4 outputs per 6-point
tile, 6 m-phases) cuts m-columns/output to 1.5 (vs 2 for F(2,3), 4 for
direct), and the epilogue applies the 6->4 inverse transform
(out0=M0+S+S2, out1=D+2D2, out2=S+4S2, out3=D+8D2+M5 with S/D =
M1+-M2, S2/D2 = M3+-M4) on ACT+DVE.

Layout per (phase k, slab): m_k[ci, X(18), Y(18), zt(4)], col =
X*72 + Y*4 + zt.  HBM ships only the 64-row m_k; the 128-row dup tile
(rows 64-127 = m_k, rows 0-63 = m_k shifted +4 cols = one y-row) is
built on-chip with a SBUF->SBUF DMA, halving HBM input bytes.  A K=128
matmul at column q then covers (dy, dy+1) pairs; dy=2 singles run K=64
4-way packed (tile A on quadrant (0,0) reading lo rows, tile B on
(64,64) reading hi rows, streaming concurrently).

Tiles: out x-range of 4 planes per tile, two tiles (x0, x0+4) in PE
col-groups 0/64 -> 8 tile-pairs (4 w x 2 x-halves), N=256 streams
(4x*16y*4zt), 6 phase-accumulations in one 3-bank PSUM tile per
tile-pair (bufs=2).  Loads are column-pieced (x-planes 0-9 / 10-17):
HBM loads on the sync ring, weights + dup copies on the gpsimd ring,
both in need order; tile-pair loop is x-half-major.
"""

import os
import sys

import numpy as np

if "/opt/trn_rl_repo" not in sys.path:
    sys.path.insert(0, "/opt/trn_rl_repo")
os.environ.setdefault("JAX_PLATFORMS", "axon,cpu")

B, CIN, COUT = 2, 64, 64
S = 18          # input spatial per dim
SO = 16         # output spatial per dim
NW = 4          # output w per core
NSLAB = 6       # input w slabs per core
NPH = 6         # Winograd F(4,3) m-phases
ZT = 4          # z-tiles (of 4) per output
XPL = S * ZT              # 72 cols per x-plane
DC = S * XPL              # 1296 m-cols per slab
DKC = DC + 4              # cols incl the +4 (one y-row) dup shift
DSL = DKC + 16            # sbuf cols incl rearrange-view slack
P01 = 10 * XPL            # piece split: x-planes 0-9 | 10-17

_CACHE = {}

# F(4,3) transform matrices, interpolation points [0, 1, -1, 2, -2, inf]
_AT = np.array(
    [[1, 1, 1, 1, 1, 0],
     [0, 1, -1, 2, -2, 0],
     [0, 1, 1, 4, 4, 0],
     [0, 1, -1, 8, -8, 1]], dtype=np.float64)
_G = np.array(
    [[1 / 4, 0, 0],
     [-1 / 6, -1 / 6, -1 / 6],
     [-1 / 6, 1 / 6, -1 / 6],
     [1 / 24, 1 / 12, 1 / 6],
     [1 / 24, -1 / 12, 1 / 6],
     [0, 0, 1]], dtype=np.float64)
_BT = np.array(
    [[4, 0, -5, 0, 1, 0],
     [0, -4, -4, 1, 1, 0],
     [0, 4, -4, -1, 1, 0],
     [0, -2, -1, 2, 1, 0],
     [0, 2, -1, -2, 1, 0],
     [0, 4, 0, -5, 0, 1]], dtype=np.float64)


def _build_nc(dt_in):
    import concourse.bass as bass
    import concourse.mybir as mybir

    f32 = mybir.dt.float32

    nc = bass.Bass()
    d_h = [
        nc.dram_tensor(f"d{k}", [64, NSLAB, DKC], dt_in, kind="ExternalInput")
        for k in range(NPH)
    ]
    # wp{k}: pair weights, lo rows = Gw_k[(dw,dx), dy=0], hi rows = dy=1
    # ws{k}: single weights, both halves = Gw_k[(dw,dx), dy=2]
    wp_h = [
        nc.dram_tensor(f"wp{k}", [128, 9, COUT], dt_in, kind="ExternalInput")
        for k in range(NPH)
    ]
    ws_h = [
        nc.dram_tensor(f"ws{k}", [128, 9, COUT], dt_in, kind="ExternalInput")
        for k in range(NPH)
    ]
    out_h = nc.dram_tensor(
        "out", [COUT, NW, SO, SO, SO], f32, kind="ExternalOutput"
    )

    tc = _make_tile_context(nc)
    with tc:
        with (
            tc.tile_pool(name="xp", bufs=1) as xpool,
            tc.tile_pool(name="wpl", bufs=1) as wpool,
            tc.tile_pool(name="ob", bufs=3) as opool,
            tc.tile_pool(name="sc", bufs=2) as spool,
            tc.tile_pool(name="ps", bufs=2, space="PSUM") as ppool,
        ):
            # scalar (ACT) / vector (DVE) issue no DMA: trigger
            # instructions stall on DMA-ring backpressure in the engine
            # FIFO, which would delay the epilogue ops behind them.
            wps, wss = [], []
            for k in range(NPH):
                wp_t = wpool.tile([128, 9, COUT], dt_in, tag=f"wp{k}")
                ws_t = wpool.tile([128, 9, COUT], dt_in, tag=f"ws{k}")
                wps.append(wp_t)
                wss.append(ws_t)
            ds = []
            for k in range(NPH):
                row = []
                for s in range(NSLAB):
                    d_t = xpool.tile([128, DSL], dt_in, tag=f"d{k}s{s}")
                    row.append(d_t)
                ds.append(row)

            def load_piece(k, s, c0, c1):
                # HBM load (sync ring) of m_k cols [c0,c1) into hi rows,
                # then on-chip dup (gpsimd ring): lo rows = m_k shifted +4
                nc.sync.dma_start(
                    ds[k][s][64:128, c0:c1], d_h[k][:, s, c0:c1]
                )
                nc.gpsimd.dma_start(
                    ds[k][s][0:64, c0 + 4 : c1 + 4],
                    ds[k][s][64:128, c0:c1],
                )

            # need-ordered loads: the first tile-pair consumes phases in
            # order, each reading slabs 0-2 of its k, so interleave
            # [weights_k, m_k slabs 0-2 piece01] per k, then the w>0
            # slabs, then the second x-half pieces.
            for k in range(NPH):
                nc.gpsimd.dma_start(wps[k][:], wp_h[k][:])
                nc.gpsimd.dma_start(wss[k][:], ws_h[k][:])
                for s in range(3):
                    load_piece(k, s, 0, P01)
            for s in range(3, NSLAB):
                for k in range(NPH):
                    load_piece(k, s, 0, P01)
            for s in range(NSLAB):
                for k in range(NPH):
                    load_piece(k, s, P01, DC)

            # HAM warmup: the PE clock-gate runs cold (1.2 GHz) until
            # ~3.4us of sustained matmul activity.  Dependency-free
            # matmuls on never-written scratch warm it up while the
            # engine preamble + first loads land (~8-9us), so the real
            # stream starts near 2.4 GHz.
            warm_ps = ppool.tile([128, NPH, 256], f32, tag="ps")
            wscr = xpool.tile([128, 640], dt_in, tag="wscr")
            nc.vector.memset(wscr[:], 0.5)
            for _ in range(5):
                nc.tensor.matmul(
                    warm_ps[:, 0][0:64, :],
                    wscr[:, 0:64],
                    wscr[:, 64:320],
                    start=True, stop=True,
                    tile_position=(0, 0),
                )

            def rhs(t, prange, q0, nx=4):
                v = t[prange, q0 : q0 + nx * XPL]
                v = v.rearrange("p (x y z) -> p x y z", x=nx, y=S, z=ZT)
                return v[:, :, 0:16, :]

            PFULL = slice(0, 128)
            PLO = slice(0, 64)
            PHI = slice(64, 128)

            def emit_phase(pk, k, w, x0):
                # dy (0,1) pairs: 9 K=128 matmuls per tile
                for j2 in range(9):
                    dw, dx = j2 // 3, j2 % 3
                    dt_ = ds[k][w + dw]
                    q = (x0 + dx) * XPL + 4
                    st = j2 == 0
                    nc.tensor.matmul(
                        pk[0:64, :],
                        wps[k][:, j2, :],
                        rhs(dt_, PFULL, q),
                        start=st, stop=False,
                        tile_position=(0, 0),
                    )
                    nc.tensor.matmul(
                        pk[64:128, :],
                        wps[k][:, j2, :],
                        rhs(dt_, PFULL, q + 4 * XPL),
                        start=st, stop=False,
                        tile_position=(0, 64),
                    )
                # dy=2 singles: K=64, 4-way packed (A lo / B hi)
                for j2 in range(9):
                    dw, dx = j2 // 3, j2 % 3
                    dt_ = ds[k][w + dw]
                    last = j2 == 8
                    nc.tensor.matmul(
                        pk[0:64, :],
                        wss[k][0:64, j2, :],
                        rhs(dt_, PLO, (x0 + dx) * XPL + 12),
                        start=False, stop=last,
                        tile_position=(0, 0),
                    )
                    nc.tensor.matmul(
                        pk[64:128, :],
                        wss[k][64:128, j2, :],
                        rhs(dt_, PHI, (x0 + 4 + dx) * XPL + 8),
                        start=False, stop=last,
                        tile_position=(64, 64),
                    )

            AOP = mybir.AluOpType

            def emit_epilogue(ps, osb, sc):
                # F(4,3) inverse along z; one PSUM operand per DVE op.
                M = [ps[:, i, :] for i in range(NPH)]
                Ssum, D, S2, D2 = (sc[:, :, i] for i in range(4))
                o0, o1, o2, o3 = (osb[:, :, i] for i in range(4))
                nc.scalar.copy(Ssum, M[1])
                nc.vector.tensor_add(out=Ssum, in0=M[2], in1=Ssum)
                nc.scalar.copy(D, M[1])
                nc.vector.scalar_tensor_tensor(
                    out=D, in0=M[2], scalar=-1.0, in1=D,
                    op0=AOP.mult, op1=AOP.add,
                )
                nc.scalar.copy(S2, M[3])
                nc.vector.tensor_add(out=S2, in0=M[4], in1=S2)
                nc.scalar.copy(D2, M[3])
                nc.vector.scalar_tensor_tensor(
                    out=D2, in0=M[4], scalar=-1.0, in1=D2,
                    op0=AOP.mult, op1=AOP.add,
                )
                # out0 = M0 + S + S2
                nc.vector.tensor_add(out=o0, in0=Ssum, in1=S2)
                nc.vector.tensor_add(out=o0, in0=M[0], in1=o0)
                # out1 = D + 2*D2
                nc.vector.scalar_tensor_tensor(
                    out=o1, in0=D2, scalar=2.0, in1=D,
                    op0=AOP.mult, op1=AOP.add,
                )
                # out2 = S + 4*S2
                nc.vector.scalar_tensor_tensor(
                    out=o2, in0=S2, scalar=4.0, in1=Ssum,
                    op0=AOP.mult, op1=AOP.add,
                )
                # out3 = D + 8*D2 + M5
                nc.vector.scalar_tensor_tensor(
                    out=o3, in0=D2, scalar=8.0, in1=D,
                    op0=AOP.mult, op1=AOP.add,
                )
                nc.vector.tensor_add(out=o3, in0=M[5], in1=o3)

            def emit_stores(osb, w, x0):
                lo = osb[0:64].rearrange(
                    "p (x y zt) r -> p x y (zt r)", x=4, y=16, zt=ZT
                )
                hi = osb[64:128].rearrange(
                    "p (x y zt) r -> p x y (zt r)", x=4, y=16, zt=ZT
                )
                # stores on the HWDGE (sync) ring only: SWDGE stores
                # would hold up the gpsimd teardown drain ~2us.
                for c0 in (0, 2):
                    nc.sync.dma_start(
                        out_h[:, w, x0 + c0 : x0 + c0 + 2, :, :],
                        lo[:, c0 : c0 + 2],
                    )
                    nc.sync.dma_start(
                        out_h[:, w, x0 + 4 + c0 : x0 + 6 + c0, :, :],
                        hi[:, c0 : c0 + 2],
                    )

            # ---- main loop: 8 tile-pairs, x-half-major ----
            for xh in range(2):
                x0 = 8 * xh       # tile A covers x-planes x0..x0+3
                for w in range(NW):
                    ps = ppool.tile([128, NPH, 256], f32, tag="ps")
                    osb = opool.tile([128, 256, 4], f32, tag="osb")
                    sc = spool.tile([128, 256, 4], f32, tag="sc")
                    for k in range(NPH):
                        emit_phase(ps[:, k, :], k, w, x0)
                    emit_epilogue(ps, osb, sc)
                    emit_stores(osb, w, x0)

    _split_multiwaits(nc)
    return nc


def _make_tile_context(nc):
    from concourse.tile import TileContext

    class TC(TileContext):
        # stock teardown is drain -> barrier -> sem-clear -> barrier; the
        # final barrier only orders engine-stream ends and costs ~2us.
        def _drain_and_barrier(self, tick_clock, wait_clock):
            from concourse.vector_clock import ScopedClock

            nc = self.nc
            drain_inst = nc.sync.drain()
            wait_clock.add_sem_waits(
                drain_inst.ins, ScopedClock({None: tick_clock.global_clock})
            )
            # mark for _split_multiwaits: distribute this drain's waits
            # round-robin across engines (parallel NoOps) instead of ~60
            # serial NoOps on sync (~1.5us tail).  The barrier right
            # after orders every NoOp before the sem clear.  gpsimd is
            # excluded: wait-NoOps there would delay its SWDGE drain
            # (~2.7us of queue teardown) past the barrier, exposing it.
            nc._final_drain_name = drain_inst.ins.name
            nc.all_engine_barrier()
            assert self.sems is not None
            popped = nc._tile_sem_poison_stack.pop()
            assert popped is self._sem_poison
            nc.clear_and_free_semaphores(list(self.sems.allocated().values()))

    return TC(nc)


def _split_multiwaits(nc, max_waits=1):
    """The walrus build here rejects any instruction carrying more than one
    sync-wait ("Too many sync wait commands").  Tile attaches one wait per
    outstanding producer.  Move excess waits onto NoOps inserted
    immediately before the instruction - same-engine, except for the
    teardown drain whose waits are spread round-robin across engines."""
    import concourse.mybir as mybir

    final_drain = getattr(nc, "_final_drain_name", None)
    # EngineType.Pool is the gpsimd queue - excluded (see teardown note)
    spread_engines = [
        e for e in nc.engines if e != mybir.EngineType.Pool
    ] or list(nc.engines)

    n_split = 0
    for fn in nc.m.functions:
        for blk in fn.blocks:
            out = []
            for inst in list(blk.instructions):
                si = inst.sync_info
                if si is not None and si.on_wait and len(si.on_wait) > max_waits:
                    waits = list(si.on_wait)
                    extra = waits[:-max_waits]
                    spread = inst.name == final_drain
                    for k in range(0, len(extra), max_waits):
                        nop = mybir.InstNoOp(
                            name=f"{inst.name}.w{k}", ins=[], outs=[]
                        )
                        if spread:
                            nop.engine = spread_engines[
                                (k // max_waits) % len(spread_engines)
                            ]
                        else:
                            nop.engine = inst.engine
                        nop.sync_info = mybir.SyncInfo(
                            on_wait=extra[k : k + max_waits], on_update=[]
                        )
                        nc.register_instruction(nop)
                        out.append(nop)
                        n_split += 1
                    si.on_wait = waits[-max_waits:]
                out.append(inst)
            blk.instructions = out
    return n_split


# compute dtype: "float16" (fastest, rel err ~1e-3) or "float32r"
DTYPE = "float16"


def _get_nc():
    if "nc" not in _CACHE:
        import concourse.mybir as mybir

        _CACHE["nc"] = _build_nc(getattr(mybir.dt, DTYPE))
    return _CACHE["nc"]


def _np_dtype():
    if DTYPE == "float16":
        return np.float16
    return np.float32


def _shard_inputs(inputs):
    nd = _np_dtype()
    x = np.asarray(inputs["inputs"], dtype=np.float32)
    wk = np.asarray(inputs["kernel"], dtype=np.float32)
    k5 = wk.reshape(3, 3, 3, 3, CIN, COUT)  # [dw, dx, dy, dz, ci, co]
    # weight transform Gw_k over dz
    gw = np.einsum("ij,wxyjcd->iwxycd", _G, k5.astype(np.float64))
    wps, wss = [], []
    for k in range(NPH):
        g = gw[k].reshape(9, 3, CIN, COUT)  # [(dw,dx), dy, ci, co]
        wp = np.concatenate(
            [g[:, 0].transpose(1, 0, 2), g[:, 1].transpose(1, 0, 2)], axis=0
        )
        w2h = g[:, 2].transpose(1, 0, 2)
        ws_ = np.concatenate([w2h, w2h], axis=0)
        wps.append(np.ascontiguousarray(wp.astype(nd)))
        wss.append(np.ascontiguousarray(ws_.astype(nd)))
    in_maps = []
    for c in range(8):
        b, wc = c // 4, c % 4
        w0c = 4 * wc
        sl = x[b, :, w0c : w0c + 6]             # [CIN, 6, 18, 18, 18] fp32
        # z windows of 6, stride 4: zt = 0..3
        xw = np.stack(
            [sl[..., 4 * t : 4 * t + 6] for t in range(ZT)], axis=-2
        )                                        # [CIN, 6, 18, 18, zt, j]
        m = np.einsum("ij,cswxtj->icswxt", _BT, xw)  # [6, CIN, 6, 18, 18, zt]
        feeds = {}
        for k in range(NPH):
            mk = m[k].reshape(CIN, NSLAB, DC).astype(nd)
            dk = np.zeros((CIN, NSLAB, DKC), dtype=nd)
            dk[:, :, 0:DC] = mk
            feeds[f"d{k}"] = dk
            feeds[f"wp{k}"] = wps[k]
            feeds[f"ws{k}"] = wss[k]
        in_maps.append(feeds)
    return in_maps


def _gather_outputs(results):
    out = np.empty((B, COUT, NW * 4, SO, SO, SO), dtype=np.float32)
    for c in range(8):
        b, wc = c // 4, c % 4
        w0 = 4 * wc
        out[b, :, w0 : w0 + 4] = results[c]["out"]
    return out


def kernel(**inputs):
    from concourse.bass_utils import run_bass_kernel_spmd

    res = run_bass_kernel_spmd(_get_nc(), _shard_inputs(inputs), list(range(8)))
    return _gather_outputs(res.results)
